# revision 29
# baseline (speedup 1.0000x reference)
"""Trainium2 Bass kernel for Convpass-swin hypernet fused adapter.

Reference computation (per batch sample):
  h      = relu(x @ Wm1 + bm1)                    # [B,H,W,64]
  prompt = mean_hw(h) @ Wm2 + bm2                 # [B,64]  (mean commutes with matmul)
  wflat  = (emb + prompt) @ Wh + bh               # [B,96*96*9]
  xd     = quickgelu(x @ Wd + bd)                 # [B,H,W,96]
  y      = quickgelu(conv3x3(xd, wflat))          # per-sample dynamic grouped conv
  out    = y @ Wu + bu                            # [B,H,W,384]

Sharding: data-parallel over batch B=64 across 8 cores (8 samples/core),
weights replicated.

Key structure (all host-side prep is x-independent weight folding plus pure
dtype/layout casts of x):
  * x is uploaded pre-transposed as [C_part=128, 3, BL, P] fp16, so no
    on-chip transposes are needed; meta/adapter matmuls read it directly.
  * The hypernet constant term w0 = (emb+bm2)@Wh + bh is folded into an
    extra row of Wh; the device computes wflat = prompt_raw @ Wh + w0 via an
    augmented [65,BL] pvec in fp16.
  * Wh is permuted TAP-MAJOR (t,o,i): while the 10.8 MB matrix streams from
    HBM, five (sample,half) conv accumulations live in persistent PSUM banks
    and consume each tap as soon as its weights land, hiding a quarter of
    the conv behind the DMA stream. The remaining conv halves and all
    output projections run after the stream, software-pipelined through the
    freed PSUM banks (convs rotate tiles 0-1, projections 2-4), with
    out-halves interleaved between conv-halves so the PSUM->SBUF copy ring
    never gates the PE.
  * The output projection runs with C on partitions (stationary Wu^T
    chunks), so bu is a per-partition bias on the PSUM->SBUF copy and the
    output goes back as [C_part, 3, BL, P] fp16 (host transposes back).
"""
import numpy as np

import concourse.bass as bass
import concourse.tile as tile
import concourse.mybir as mybir
from concourse import bacc
from concourse.bass_utils import run_bass_kernel_spmd

F32 = mybir.dt.float32
F16 = mybir.dt.float16
AF = mybir.ActivationFunctionType
AX = mybir.AxisListType

# problem constants
B, H, W, C = 64, 28, 28, 384
DIM, E, KK = 96, 64, 3
NCORES = 8
BL = B // NCORES          # samples per core
P = H * W                 # 784 positions per sample
HP = H + 2                # padded spatial
WH_COLS = DIM * DIM * 9   # 82944
NCH = 27                  # Wh stream chunks
CHW = WH_COLS // NCH      # 3072 columns per chunk
GRP = CHW // DIM          # 32 (t,o) groups per chunk
HF = P // 2               # 392 positions per half
NRES = 3                  # first samples with stream-resident conv accumulators


def build_nc():
    nc = bacc.Bacc("TRN2", target_bir_lowering=False, debug=False)

    xt_d = nc.dram_tensor("xt", [128, 3, BL, P], F16, kind="ExternalInput").ap()
    w16_d = nc.dram_tensor("w16", [128, 936], F16, kind="ExternalInput").ap()
    bias_d = nc.dram_tensor("bias", [128, 5], F32, kind="ExternalInput").ap()
    wh_d = nc.dram_tensor("wh", [E + 1, WH_COLS], F16, kind="ExternalInput").ap()
    yo_d = nc.dram_tensor("yo", [128, 3, BL, P], F16, kind="ExternalOutput").ap()

    with tile.TileContext(nc) as tc:
        with (
            tc.tile_pool(name="const", bufs=1) as cp,
            tc.tile_pool(name="persist", bufs=1) as pp,
            tc.tile_pool(name="scr", bufs=3) as sp,
            tc.tile_pool(name="wh", bufs=16) as wh_p,
            tc.tile_pool(name="ob", bufs=4) as ob_p,
        ):
            # ---- constants (two DMAs), then the x stream ----
            w16_sb = cp.tile([128, 936], F16)
            nc.sync.dma_start(w16_sb[:], w16_d[:])
            bias_sb = cp.tile([128, 5], F32)
            nc.sync.dma_start(bias_sb[:], bias_d[:])

            xt_sb = pp.tile([128, 3, BL, P], F16)
            for b in range(BL):
                nc.sync.dma_start(xt_sb[:, :, b, :], xt_d[:, :, b, :])

            # ---- persistent state ----
            xd_pad = pp.tile([DIM, BL, HP, HP], F16)      # padded gelu(x@Wd+bd)
            w_all = pp.tile([DIM, BL, 9, DIM], F16)       # per-sample conv weights [i,b,t,o]
            y_all = pp.tile([DIM, BL, P], F16)            # gelu(conv) activations
            hsum = pp.tile([E, BL, 2], F32)               # per-(sample,half) relu sums
            hbar = pp.tile([E, BL], F32)
            hbar16 = pp.tile([E, BL], F16)
            pvec = pp.tile([E + 1, BL], F16)              # (prompt_raw, 1)^T

            # zero only the conv halo ring; phase 1b writes the interior
            nc.vector.memset(xd_pad[:, :, 0:1, :], 0.0)
            nc.vector.memset(xd_pad[:, :, HP - 1:HP, :], 0.0)
            nc.vector.memset(xd_pad[:, :, 1:HP - 1, 0:1], 0.0)
            nc.vector.memset(xd_pad[:, :, 1:HP - 1, HP - 1:HP], 0.0)
            nc.vector.memset(pvec[E:E + 1, :], 1.0)

            # pwg pool opens first so pool releases stay LIFO
            ps2 = tc.tile_pool(name="ps2", bufs=1, space="PSUM")
            p2 = ps2.__enter__()
            w_gb = w_all[:].rearrange("i b t o -> i (t o) b")  # [96, 864, 8], tap-major

            # ---- phase 1a: meta h = relu(x@Wm1+bm1), spatial sums ----
            ps1 = tc.tile_pool(name="ps1", bufs=1, space="PSUM")
            p1 = ps1.__enter__()
            for b in range(BL):
                for h2 in range(2):
                    ph = p1.tile([E, HF], F32, name="ph", tag="ph", bufs=2)
                    for c in range(3):
                        nc.tensor.matmul(ph[:], w16_sb[:, c * 64:(c + 1) * 64],
                                         xt_sb[:, c, b, h2 * HF:(h2 + 1) * HF],
                                         start=(c == 0), stop=(c == 2))
                    h_scr = sp.tile([E, HF], F32, tag="hscr")
                    nc.scalar.activation(h_scr[:], ph[:], AF.Relu,
                                         bias=bias_sb[0:E, 0:1],
                                         accum_out=hsum[:, b, h2:h2 + 1])
            nc.vector.reduce_sum(hbar[:], hsum[:], axis=AX.X)
            nc.scalar.activation(hbar16[:], hbar[:], AF.Copy, scale=1.0 / P)

            # ---- phase 1b + prompt + Wh stream start, interleaved ----
            def emit_xd(b):
                for h2 in range(2):
                    px = p1.tile([DIM, HF], F32, name="px", tag="px", bufs=2)
                    for c in range(3):
                        nc.tensor.matmul(px[:], w16_sb[:, 192 + c * 96:192 + (c + 1) * 96],
                                         xt_sb[:, c, b, h2 * HF:(h2 + 1) * HF],
                                         start=(c == 0), stop=(c == 2))
                    nc.scalar.activation(
                        xd_pad[:, b, 1 + h2 * 14:15 + h2 * 14, 1:29],
                        px[:].rearrange("p (r c) -> p r c", r=14),
                        AF.Gelu_apprx_sigmoid, bias=bias_sb[0:DIM, 1:2])

            emit_xd(0)
            ppm = p1.tile([E, BL], F32, name="ppm", tag="ppm", bufs=1)
            nc.tensor.matmul(ppm[:], w16_sb[0:E, 480:544], hbar16[:],
                             start=True, stop=True)
            nc.scalar.activation(pvec[0:E, :], ppm[:], AF.Copy)

            def emit_ph3(jd):
                whc = wh_p.tile([E + 1, CHW], F16, tag="whc")
                nc.sync.dma_start(whc[:], wh_d[:, jd * CHW:(jd + 1) * CHW])
                pwg = p2.tile([DIM, GRP * BL], F32, name="pwg", tag="pwg", bufs=3)
                for gg in range(GRP):
                    nc.tensor.matmul(pwg[:, gg * BL:(gg + 1) * BL],
                                     whc[:, gg * DIM:(gg + 1) * DIM], pvec[:],
                                     start=True, stop=True)
                g0 = jd * GRP
                src = pwg[:].rearrange("i (g b) -> i g b", g=GRP)
                if jd % 2 == 0:
                    nc.vector.tensor_copy(w_gb[:, g0:g0 + GRP, :], src)
                else:
                    nc.scalar.activation(w_gb[:, g0:g0 + GRP, :], src, AF.Copy)

            jd = 0
            for b in range(1, BL):
                emit_xd(b)
                emit_ph3(jd); emit_ph3(jd + 1)
                jd += 2
            ps1.__exit__(None, None, None)

            # ---- resident conv: taps accumulate while Wh streams ----
            res = tc.tile_pool(name="res", bufs=1, space="PSUM")
            pr = res.__enter__()
            RES_PAIRS = [(0, 0), (0, 1), (1, 0), (1, 1), (2, 0)]
            py_res = [pr.tile([128, HF], F32, name=f"pyr{i}")
                      for i in range(len(RES_PAIRS))]

            def emit_res_tap(t):
                for i, (b, h2) in enumerate(RES_PAIRS):
                    dy, dx = t // 3, t % 3
                    nc.tensor.matmul(
                        py_res[i][0:DIM, :], w_all[:, b, t, :],
                        xd_pad[:, b, h2 * 14 + dy:h2 * 14 + dy + 14, dx:dx + 28],
                        start=(t == 0), stop=(t == 8))

            tap_next = 0
            # copies are emitted through chunk jd when jd is odd (pair flush)
            # or at the final chunk; taps may only consume copied weights
            def emit_ready_taps(jc):
                nonlocal tap_next
                while tap_next < 9 and 3 * tap_next + 2 <= jc:
                    emit_res_tap(tap_next)
                    tap_next += 1

            emit_ready_taps(jd - 5)
            while jd < NCH:
                emit_ph3(jd)
                if jd % 2 == 1 or jd == NCH - 1:
                    # two-pair emission lag so the PE never stalls on a copy
                    emit_ready_taps(jd - 4)
                jd += 1
            emit_ready_taps(NCH - 1)
            for i, (b, h2) in enumerate(RES_PAIRS):
                nc.scalar.activation(y_all[:, b, h2 * HF:(h2 + 1) * HF],
                                     py_res[i][0:DIM, :], AF.Gelu_apprx_sigmoid)

            # ---- tail: remaining convs + output projection for all samples ----
            # non-resident convs rotate through the resident PSUM tiles (the
            # write only waits for that tile's gelu read, not the whole pool)
            rot = 0
            pyr_rot = [0]

            def emit_out_half(b, ob, h2):
                nonlocal rot
                for c in range(3):
                    po = py_res[2 + po_rot[0] % 3]
                    po_rot[0] += 1
                    nc.tensor.matmul(po[:], w16_sb[0:DIM, 552 + c * 128:552 + (c + 1) * 128],
                                     y_all[:, b, h2 * HF:(h2 + 1) * HF],
                                     start=True, stop=True)
                    dst = ob[:, c, h2, :]
                    bcol = bias_sb[:, 2 + c:3 + c]
                    if rot % 2 == 0:
                        nc.scalar.activation(dst, po[:], AF.Identity, bias=bcol)
                    else:
                        nc.vector.tensor_scalar_add(dst, po[:], bcol)
                    rot += 1
                nc.sync.dma_start(yo_d[:, :, b, h2 * HF:(h2 + 1) * HF],
                                  ob[:, :, h2, :])

            po_rot = [0]

            def emit_conv_half(b, h2):
                py = py_res[pyr_rot[0] % 2]
                pyr_rot[0] += 1
                for t in range(9):
                    dy, dx = t // 3, t % 3
                    nc.tensor.matmul(
                        py[0:DIM, :], w_all[:, b, t, :],
                        xd_pad[:, b, h2 * 14 + dy:h2 * 14 + dy + 14, dx:dx + 28],
                        start=(t == 0), stop=(t == 8))
                nc.scalar.activation(y_all[:, b, h2 * HF:(h2 + 1) * HF],
                                     py[0:DIM, :], AF.Gelu_apprx_sigmoid)

            # software pipeline: out-halves interleave between conv-halves so
            # the projection's psum ring and copies never gate the PE
            conv_halves = [(2, 1)] + [(b, h2) for b in range(NRES, BL)
                                      for h2 in range(2)]
            out_halves = [(b, h2) for b in range(BL) for h2 in range(2)]
            oq = 0
            obs = {}

            def emit_next_out():
                nonlocal oq
                b, h2 = out_halves[oq]
                if h2 == 0:
                    obs[b] = ob_p.tile([128, 3, 2, HF], F16, name=f"ob{b}", tag="ob")
                emit_out_half(b, obs[b], h2)
                oq += 1

            for b, h2 in conv_halves:
                emit_conv_half(b, h2)
                if oq < len(out_halves):
                    emit_next_out()
            while oq < len(out_halves):
                emit_next_out()
            res.__exit__(None, None, None)
            ps2.__exit__(None, None, None)

    nc.compile()
    return nc


_NC_CACHE = None


def _get_nc():
    global _NC_CACHE
    if _NC_CACHE is None:
        _NC_CACHE = build_nc()
    return _NC_CACHE


def _prep_inputs(x, Wd, bd, Wm1, bm1, Wm2, bm2, Wh, bh, emb, Wu, bu):
    """Host-side prep: pure layout/dtype transforms + x-independent weight
    folding (w0 row, bias packing)."""
    f16 = np.float16
    Wh = np.asarray(Wh, np.float32)
    w0 = (np.asarray(emb, np.float32) + np.asarray(bm2, np.float32)) @ Wh \
        + np.asarray(bh, np.float32)
    # (e, o, i, t) -> tap-major (e, t, o, i)
    whp = Wh.reshape(E, DIM, DIM, KK * KK).transpose(0, 3, 1, 2).reshape(E, WH_COLS)
    w0p = w0.reshape(DIM, DIM, KK * KK).transpose(2, 0, 1).reshape(1, WH_COLS)
    wh_aug = np.concatenate([whp, w0p], 0).astype(f16)

    w16 = np.zeros((128, 936), f16)
    w16[:, 0:192] = np.asarray(Wm1, np.float32).reshape(3, 128, E) \
        .transpose(1, 0, 2).reshape(128, 192)
    w16[:, 192:480] = np.asarray(Wd, np.float32).reshape(3, 128, DIM) \
        .transpose(1, 0, 2).reshape(128, 288)
    w16[0:E, 480:544] = np.asarray(Wm2, np.float32)
    w16[0:DIM, 552:936] = np.asarray(Wu, np.float32)

    bias = np.zeros((128, 5), np.float32)
    bias[0:E, 0] = np.asarray(bm1, np.float32)
    bias[0:DIM, 1] = np.asarray(bd, np.float32)
    bias[:, 2:5] = np.asarray(bu, np.float32).reshape(3, 128).T

    shared = {"w16": w16, "bias": bias, "wh": wh_aug}
    xs = np.asarray(x, np.float32).astype(f16).reshape(B, P, C)
    in_maps = []
    for k in range(NCORES):
        xt = xs[k * BL:(k + 1) * BL].reshape(BL, P, 3, 128).transpose(3, 2, 0, 1)
        m = dict(shared)
        m["xt"] = np.ascontiguousarray(xt)
        in_maps.append(m)
    return in_maps


def _run(inputs, **spmd_kwargs):
    nc = _get_nc()
    in_maps = _prep_inputs(**inputs)
    res = run_bass_kernel_spmd(nc, in_maps, core_ids=list(range(NCORES)), **spmd_kwargs)
    parts = []
    for r in res.results:
        yo = np.asarray(r["yo"])                      # [128, 3, BL, P] fp16
        parts.append(yo.transpose(2, 3, 1, 0).reshape(BL, P, C))
    out = np.concatenate(parts, 0).astype(np.float32)
    return out.reshape(B, H, W, C), res


def kernel(**inputs) -> np.ndarray:
    out, _ = _run(inputs)
    return out


# revision 41
# speedup vs baseline: 1.0241x; 1.0241x over previous
"""Trainium2 Bass kernel for Convpass-swin hypernet fused adapter.

Reference computation (per batch sample):
  h      = relu(x @ Wm1 + bm1)                    # [B,H,W,64]
  prompt = mean_hw(h) @ Wm2 + bm2                 # [B,64]  (mean commutes with matmul)
  wflat  = (emb + prompt) @ Wh + bh               # [B,96*96*9]
  xd     = quickgelu(x @ Wd + bd)                 # [B,H,W,96]
  y      = quickgelu(conv3x3(xd, wflat))          # per-sample dynamic grouped conv
  out    = y @ Wu + bu                            # [B,H,W,384]

Sharding: data-parallel over batch B=64 across 8 cores (8 samples/core),
weights replicated.

Key structure (all host-side prep is x-independent weight folding plus pure
dtype/layout casts of x):
  * x is uploaded pre-transposed as [C_part=128, 3, BL, P] fp16, so no
    on-chip transposes are needed; meta/adapter matmuls read it directly.
  * The hypernet constant term w0 = (emb+bm2)@Wh + bh is folded into an
    extra row of Wh; the device computes wflat = prompt_raw @ Wh + w0 via an
    augmented [65,BL] pvec in fp16.
  * Wh is permuted TAP-MAJOR (t,o,i): while the 10.8 MB matrix streams from
    HBM, five (sample,half) conv accumulations live in persistent PSUM banks
    and consume each tap as soon as its weights land, hiding a quarter of
    the conv behind the DMA stream. The remaining conv halves and all
    output projections run after the stream, software-pipelined through the
    freed PSUM banks (convs rotate tiles 0-1, projections 2-4), with
    out-halves interleaved between conv-halves so the PSUM->SBUF copy ring
    never gates the PE.
  * The output projection runs with C on partitions (stationary Wu^T
    chunks), so bu is a per-partition bias on the PSUM->SBUF copy and the
    output goes back as [C_part, 3, BL, P] fp16 (host transposes back).
"""
import numpy as np

import concourse.bass as bass
import concourse.tile as tile
import concourse.mybir as mybir
from concourse import bacc
from concourse.bass_utils import run_bass_kernel_spmd

F32 = mybir.dt.float32
F16 = mybir.dt.float16
AF = mybir.ActivationFunctionType
AX = mybir.AxisListType

# problem constants
B, H, W, C = 64, 28, 28, 384
DIM, E, KK = 96, 64, 3
NCORES = 8
BL = B // NCORES          # samples per core
P = H * W                 # 784 positions per sample
HP = H + 2                # padded spatial
WH_COLS = DIM * DIM * 9   # 82944
NCH = 27                  # Wh stream chunks
CHW = WH_COLS // NCH      # 3072 columns per chunk
GRP = CHW // DIM          # 32 (t,o) groups per chunk
HF = P // 2               # 392 positions per half
NRES = 3                  # first samples with stream-resident conv accumulators


def build_nc():
    nc = bacc.Bacc("TRN2", target_bir_lowering=False, debug=False)

    xt_d = nc.dram_tensor("xt", [128, 3, BL, P], F16, kind="ExternalInput").ap()
    w16_d = nc.dram_tensor("w16", [128, 936], F16, kind="ExternalInput").ap()
    bias_d = nc.dram_tensor("bias", [128, 5], F32, kind="ExternalInput").ap()
    wh_d = nc.dram_tensor("wh", [E + 1, WH_COLS], F16, kind="ExternalInput").ap()
    yo_d = nc.dram_tensor("yo", [128, 3, BL, P], F16, kind="ExternalOutput").ap()

    with tile.TileContext(nc) as tc:
        with (
            tc.tile_pool(name="const", bufs=1) as cp,
            tc.tile_pool(name="persist", bufs=1) as pp,
            tc.tile_pool(name="scr", bufs=3) as sp,
            tc.tile_pool(name="wh", bufs=16) as wh_p,
            tc.tile_pool(name="ob", bufs=4) as ob_p,
        ):
            # ---- constants (two DMAs), then the x stream ----
            w16_sb = cp.tile([128, 936], F16)
            nc.sync.dma_start(w16_sb[:], w16_d[:])

            xt_sb = pp.tile([128, 3, BL, P], F16)
            for b in range(2):
                nc.sync.dma_start(xt_sb[:, :, b, :], xt_d[:, :, b, :])
            bias_sb = cp.tile([128, 5], F32)
            nc.sync.dma_start(bias_sb[:], bias_d[:])
            for b in range(2, BL):
                nc.sync.dma_start(xt_sb[:, :, b, :], xt_d[:, :, b, :])

            # ---- persistent state ----
            xd_pad = pp.tile([DIM, BL, HP, HP], F16)      # padded gelu(x@Wd+bd)
            w_all = pp.tile([DIM, BL, 9, DIM], F16)       # per-sample conv weights [i,b,t,o]
            y_all = pp.tile([DIM, BL, P], F16)            # gelu(conv) activations
            hsum = pp.tile([E, BL, 2], F32)               # per-(sample,half) relu sums
            hbar = pp.tile([E, BL], F32)
            hbar16 = pp.tile([E, BL], F16)
            pvec = pp.tile([E + 1, BL], F16)              # (prompt_raw, 1)^T

            # zero only the conv halo ring; phase 1b writes the interior
            nc.vector.memset(xd_pad[:, :, 0:1, :], 0.0)
            nc.vector.memset(xd_pad[:, :, HP - 1:HP, :], 0.0)
            nc.vector.memset(xd_pad[:, :, 1:HP - 1, 0:1], 0.0)
            nc.vector.memset(xd_pad[:, :, 1:HP - 1, HP - 1:HP], 0.0)
            nc.vector.memset(pvec[E:E + 1, :], 1.0)

            # pwg pool opens first so pool releases stay LIFO
            ps2 = tc.tile_pool(name="ps2", bufs=1, space="PSUM")
            p2 = ps2.__enter__()
            w_gb = w_all[:].rearrange("i b t o -> i (t o) b")  # [96, 864, 8], tap-major

            # ---- phase 1a: meta h = relu(x@Wm1+bm1), spatial sums ----
            ps1 = tc.tile_pool(name="ps1", bufs=1, space="PSUM")
            p1 = ps1.__enter__()
            for b in range(BL):
                for h2 in range(2):
                    ph = p1.tile([E, HF], F32, name="ph", tag="ph", bufs=2)
                    for c in range(3):
                        nc.tensor.matmul(ph[:], w16_sb[:, c * 64:(c + 1) * 64],
                                         xt_sb[:, c, b, h2 * HF:(h2 + 1) * HF],
                                         start=(c == 0), stop=(c == 2))
                    h_scr = sp.tile([E, HF], F32, tag="hscr")
                    nc.scalar.activation(h_scr[:], ph[:], AF.Relu,
                                         bias=bias_sb[0:E, 0:1],
                                         accum_out=hsum[:, b, h2:h2 + 1])
            nc.vector.reduce_sum(hbar[:], hsum[:], axis=AX.X)
            nc.scalar.activation(hbar16[:], hbar[:], AF.Copy, scale=1.0 / P)

            # ---- phase 1b + prompt + Wh stream start, interleaved ----
            def emit_xd(b):
                for h2 in range(2):
                    px = p1.tile([DIM, HF], F32, name="px", tag="px", bufs=2)
                    for c in range(3):
                        nc.tensor.matmul(px[:], w16_sb[:, 192 + c * 96:192 + (c + 1) * 96],
                                         xt_sb[:, c, b, h2 * HF:(h2 + 1) * HF],
                                         start=(c == 0), stop=(c == 2))
                    nc.scalar.activation(
                        xd_pad[:, b, 1 + h2 * 14:15 + h2 * 14, 1:29],
                        px[:].rearrange("p (r c) -> p r c", r=14),
                        AF.Gelu_apprx_sigmoid, bias=bias_sb[0:DIM, 1:2])

            emit_xd(0)
            ppm = p1.tile([E, BL], F32, name="ppm", tag="ppm", bufs=1)
            nc.tensor.matmul(ppm[:], w16_sb[0:E, 480:544], hbar16[:],
                             start=True, stop=True)
            nc.scalar.activation(pvec[0:E, :], ppm[:], AF.Copy)

            def emit_ph3(jd):
                whc = wh_p.tile([E + 1, CHW], F16, tag="whc")
                nc.sync.dma_start(whc[:], wh_d[:, jd * CHW:(jd + 1) * CHW])
                pwg = p2.tile([DIM, GRP * BL], F32, name="pwg", tag="pwg", bufs=3)
                for gg in range(GRP):
                    nc.tensor.matmul(pwg[:, gg * BL:(gg + 1) * BL],
                                     whc[:, gg * DIM:(gg + 1) * DIM], pvec[:],
                                     start=True, stop=True)
                g0 = jd * GRP
                src = pwg[:].rearrange("i (g b) -> i g b", g=GRP)
                if jd % 2 == 0:
                    nc.vector.tensor_copy(w_gb[:, g0:g0 + GRP, :], src)
                else:
                    nc.scalar.activation(w_gb[:, g0:g0 + GRP, :], src, AF.Copy)

            jd = 0
            for b in range(1, BL):
                emit_xd(b)
                emit_ph3(jd); emit_ph3(jd + 1)
                jd += 2
            ps1.__exit__(None, None, None)

            # ---- resident conv: taps accumulate while Wh streams ----
            res = tc.tile_pool(name="res", bufs=1, space="PSUM")
            pr = res.__enter__()
            RES_PAIRS = [(0, 0), (0, 1), (1, 0), (1, 1), (2, 0)]
            py_res = [pr.tile([128, HF], F32, name=f"pyr{i}")
                      for i in range(len(RES_PAIRS))]

            def emit_res_tap(t):
                for i, (b, h2) in enumerate(RES_PAIRS):
                    dy, dx = t // 3, t % 3
                    nc.tensor.matmul(
                        py_res[i][0:DIM, :], w_all[:, b, t, :],
                        xd_pad[:, b, h2 * 14 + dy:h2 * 14 + dy + 14, dx:dx + 28],
                        start=(t == 0), stop=(t == 8))

            tap_next = 0
            # copies are emitted through chunk jd when jd is odd (pair flush)
            # or at the final chunk; taps may only consume copied weights
            def emit_ready_taps(jc):
                nonlocal tap_next
                while tap_next < 9 and 3 * tap_next + 2 <= jc:
                    emit_res_tap(tap_next)
                    tap_next += 1

            emit_ready_taps(jd - 3)
            while jd < NCH:
                emit_ph3(jd)
                if jd % 2 == 1 or jd == NCH - 1:
                    # two-pair emission lag so the PE never stalls on a copy
                    emit_ready_taps(jd - 2)
                jd += 1
            emit_ready_taps(NCH - 1)
            for i, (b, h2) in enumerate(RES_PAIRS):
                nc.scalar.activation(y_all[:, b, h2 * HF:(h2 + 1) * HF],
                                     py_res[i][0:DIM, :], AF.Gelu_apprx_sigmoid)

            # ---- tail: remaining convs + output projection for all samples ----
            # non-resident convs rotate through the resident PSUM tiles (the
            # write only waits for that tile's gelu read, not the whole pool)
            rot = 0
            pyr_rot = [0]

            def emit_out_half(b, ob, h2):
                nonlocal rot
                for c in range(3):
                    po = py_res[2 + po_rot[0] % 3]
                    po_rot[0] += 1
                    nc.tensor.matmul(po[:], w16_sb[0:DIM, 552 + c * 128:552 + (c + 1) * 128],
                                     y_all[:, b, h2 * HF:(h2 + 1) * HF],
                                     start=True, stop=True)
                    dst = ob[:, c, h2, :]
                    bcol = bias_sb[:, 2 + c:3 + c]
                    if rot % 2 == 0:
                        nc.scalar.activation(dst, po[:], AF.Identity, bias=bcol)
                    else:
                        nc.vector.tensor_scalar_add(dst, po[:], bcol)
                    rot += 1
                nc.sync.dma_start(yo_d[:, :, b, h2 * HF:(h2 + 1) * HF],
                                  ob[:, :, h2, :])

            po_rot = [0]

            def emit_conv_half(b, h2):
                py = py_res[pyr_rot[0] % 2]
                pyr_rot[0] += 1
                for t in range(9):
                    dy, dx = t // 3, t % 3
                    nc.tensor.matmul(
                        py[0:DIM, :], w_all[:, b, t, :],
                        xd_pad[:, b, h2 * 14 + dy:h2 * 14 + dy + 14, dx:dx + 28],
                        start=(t == 0), stop=(t == 8))
                nc.scalar.activation(y_all[:, b, h2 * HF:(h2 + 1) * HF],
                                     py[0:DIM, :], AF.Gelu_apprx_sigmoid)

            # software pipeline: out-halves interleave between conv-halves so
            # the projection's psum ring and copies never gate the PE
            conv_halves = [(2, 1)] + [(b, h2) for b in range(NRES, BL)
                                      for h2 in range(2)]
            out_halves = [(b, h2) for b in range(BL) for h2 in range(2)]
            oq = 0
            obs = {}
            # out-half k may only be emitted once its gelu is >=1 conv-half
            # back (ready[i] = halves safe to emit after conv_halves[i])
            ready = [5 + max(0, i - 1) for i in range(len(conv_halves))]

            def emit_next_out():
                nonlocal oq
                b, h2 = out_halves[oq]
                if h2 == 0:
                    obs[b] = ob_p.tile([128, 3, 2, HF], F16, name=f"ob{b}", tag="ob")
                emit_out_half(b, obs[b], h2)
                oq += 1

            for i, (b, h2) in enumerate(conv_halves):
                emit_conv_half(b, h2)
                want = min(ready[i], len(out_halves))
                while oq < want and oq < 3 * (i + 1):
                    emit_next_out()
            while oq < len(out_halves):
                emit_next_out()
            res.__exit__(None, None, None)
            ps2.__exit__(None, None, None)

    nc.compile()
    return nc


_NC_CACHE = None


def _get_nc():
    global _NC_CACHE
    if _NC_CACHE is None:
        _NC_CACHE = build_nc()
    return _NC_CACHE


def _prep_inputs(x, Wd, bd, Wm1, bm1, Wm2, bm2, Wh, bh, emb, Wu, bu):
    """Host-side prep: pure layout/dtype transforms + x-independent weight
    folding (w0 row, bias packing)."""
    f16 = np.float16
    Wh = np.asarray(Wh, np.float32)
    w0 = (np.asarray(emb, np.float32) + np.asarray(bm2, np.float32)) @ Wh \
        + np.asarray(bh, np.float32)
    # (e, o, i, t) -> tap-major (e, t, o, i)
    whp = Wh.reshape(E, DIM, DIM, KK * KK).transpose(0, 3, 1, 2).reshape(E, WH_COLS)
    w0p = w0.reshape(DIM, DIM, KK * KK).transpose(2, 0, 1).reshape(1, WH_COLS)
    wh_aug = np.concatenate([whp, w0p], 0).astype(f16)

    w16 = np.zeros((128, 936), f16)
    w16[:, 0:192] = np.asarray(Wm1, np.float32).reshape(3, 128, E) \
        .transpose(1, 0, 2).reshape(128, 192)
    w16[:, 192:480] = np.asarray(Wd, np.float32).reshape(3, 128, DIM) \
        .transpose(1, 0, 2).reshape(128, 288)
    w16[0:E, 480:544] = np.asarray(Wm2, np.float32)
    w16[0:DIM, 552:936] = np.asarray(Wu, np.float32)

    bias = np.zeros((128, 5), np.float32)
    bias[0:E, 0] = np.asarray(bm1, np.float32)
    bias[0:DIM, 1] = np.asarray(bd, np.float32)
    bias[:, 2:5] = np.asarray(bu, np.float32).reshape(3, 128).T

    shared = {"w16": w16, "bias": bias, "wh": wh_aug}
    xs = np.asarray(x, np.float32).astype(f16).reshape(B, P, C)
    in_maps = []
    for k in range(NCORES):
        xt = xs[k * BL:(k + 1) * BL].reshape(BL, P, 3, 128).transpose(3, 2, 0, 1)
        m = dict(shared)
        m["xt"] = np.ascontiguousarray(xt)
        in_maps.append(m)
    return in_maps


def _run(inputs, **spmd_kwargs):
    nc = _get_nc()
    in_maps = _prep_inputs(**inputs)
    res = run_bass_kernel_spmd(nc, in_maps, core_ids=list(range(NCORES)), **spmd_kwargs)
    parts = []
    for r in res.results:
        yo = np.asarray(r["yo"])                      # [128, 3, BL, P] fp16
        parts.append(yo.transpose(2, 3, 1, 0).reshape(BL, P, C))
    out = np.concatenate(parts, 0).astype(np.float32)
    return out.reshape(B, H, W, C), res


def kernel(**inputs) -> np.ndarray:
    out, _ = _run(inputs)
    return out
